# revision 8
# baseline (speedup 1.0000x reference)
"""EuclideanCodebook (VQ) kernel for 8 Trainium2 NeuronCores.

Math: argmin_k ||x - e_k||^2 == argmax_k s_k,  s_k = x.e_k - ||e_k||^2/2.
Scores are computed at ~fp32 precision on the PE via a hi/lo bf16 split:
  s = xh.eh + (b1+b2+b3) [MM1, K=67]  +  xh.el + xl.eh [MM2, K=128]
where b1,b2,b3 are bf16-exact digits of -||e||^2/2.

Argmax per 128-token tile over 4096 codes:
  - 8 matmul chunks land in PSUM [128, 512] each.
  - ACT copies chunks 4..7 to SBUF (C), DVE tensor_tensor_reduce chain
    computes max1 = max(psum_c, C_c) elementwise (pairs k, k+2048) plus
    the row max m* via chained accumulators.
  - One scalar_tensor_tensor pass: sum((max1 >= m*) * iota) = pair slot n.
  - Pair resolved by gathering e[n], e[n+2048] via indirect DMA and
    rescoring exact squared distances on DVE (also the tie-breaker).
quantize rows are gathered from the embed table by the final index.

Sharding: x split into 8 shards of 8192 tokens (one per core); embed
replicated. SPMD program via run_bass_kernel_spmd; host only
concatenates shard outputs.
"""

from contextlib import ExitStack

import numpy as np

import concourse.bass as bass
import concourse.bacc as bacc
import concourse.tile as tile
from concourse import masks, mybir
from concourse.bass import IndirectOffsetOnAxis
from concourse.bass_utils import run_bass_kernel_spmd

F32 = mybir.dt.float32
BF16 = mybir.dt.bfloat16
I32 = mybir.dt.int32
U32 = mybir.dt.uint32
OP = mybir.AluOpType
AX = mybir.AxisListType

N, K, D = 65536, 4096, 64
NCORES = 8
NS = N // NCORES          # 8192 tokens per core
NT = NS // 128            # 64 token tiles
ET_TILES = K // 128       # 32 embed tiles
NEG = -3.0e38


def build(nc: bacc.Bacc, n_tiles: int = NT):
    x = nc.dram_tensor("x", [NS, D], F32, kind="ExternalInput").ap()
    embed = nc.dram_tensor("embed", [1, K, D], F32, kind="ExternalInput").ap()
    quant = nc.dram_tensor("quant", [NS, D], F32, kind="ExternalOutput").ap()
    eidx = nc.dram_tensor("eidx", [NS, 1], I32, kind="ExternalOutput").ap()
    bscr = nc.dram_tensor("bscr", [3, K], F32).ap()
    etab = embed.rearrange("o k d -> (o k) d")  # [K, D], offset 0

    # persistent sbuf
    XF = nc.alloc_sbuf_tensor("XF", [128, NT * D], F32).ap()
    T1 = nc.alloc_sbuf_tensor("T1", [67, NS], BF16).ap()
    T2 = nc.alloc_sbuf_tensor("T2", [128, NS], BF16).ap()
    EAUG = nc.alloc_sbuf_tensor("EAUG", [67, K], BF16).ap()
    E2 = nc.alloc_sbuf_tensor("E2", [128, K], BF16).ap()
    IOTA = nc.alloc_sbuf_tensor("IOTA", [128, K // 2], F32).ap()
    IOTI = nc.alloc_sbuf_tensor("IOTI", [128, K // 2], I32).ap()
    IDENT = nc.alloc_sbuf_tensor("IDENT", [128, 128], F32).ap()

    with tile.TileContext(nc) as tc:
        masks.make_identity(nc, IDENT)
        nc.gpsimd.memset(T1[64:67, :], 1.0)
        nc.gpsimd.iota(IOTI, pattern=[[1024, 4], [2, 512]], base=0, channel_multiplier=0)
        nc.vector.tensor_copy(IOTA, IOTI)

        # ---------------- prep: embed + x (own pool scope) ----------------
        with ExitStack() as pctx:
            ep = pctx.enter_context(tc.tile_pool(name="ep", bufs=3))
            pt = pctx.enter_context(tc.tile_pool(name="pt", bufs=2, space="PSUM"))

            hcol = nc.alloc_sbuf_tensor("hcol", [128, ET_TILES], F32).ap()
            for bt in range(ET_TILES // 8):
                ps = pt.tile([128, 1024], F32, tag="ps")
                for i in range(8):
                    t = bt * 8 + i
                    etile = ep.tile([128, D], F32, tag="etile")
                    scr64 = ep.tile([128, D], F32, tag="scr64")
                    nc.sync.dma_start(etile, etab[t * 128:(t + 1) * 128, :])
                    nc.vector.tensor_tensor(out=scr64, in0=etile, in1=etile,
                                            op=OP.mult)
                    nc.vector.tensor_reduce(out=hcol[:, t:t + 1], in_=scr64,
                                            axis=AX.X, op=OP.add)
                    nc.tensor.transpose(ps[0:64, i * 128:(i + 1) * 128], etile, IDENT)
                sl = slice(bt * 1024, (bt + 1) * 1024)
                pse = ps[0:64, :]
                nc.vector.tensor_copy(EAUG[0:64, sl], pse)          # eh.T bf16
                nc.vector.tensor_tensor(out=E2[0:64, sl], in0=pse,
                                        in1=EAUG[0:64, sl], op=OP.subtract)  # el.T
                eh16 = ep.tile([64, 1024], BF16, tag="eh16")
                nc.vector.tensor_copy(eh16, pse)
                nc.sync.dma_start(E2[64:128, sl], eh16)             # partition move

            # bias digits: B = -h/2; b1 = round(B); r = B-b1;
            # b2 = round(r*256)/256; b3 = r-b2
            MAGIC = float(2 ** 23)
            Bc = ep.tile([128, ET_TILES], F32, tag="Bc")
            b1 = ep.tile([128, ET_TILES], F32, tag="b1")
            b2 = ep.tile([128, ET_TILES], F32, tag="b2")
            b3 = ep.tile([128, ET_TILES], F32, tag="b3")
            rr = ep.tile([128, ET_TILES], F32, tag="rr")
            nc.vector.tensor_scalar(out=Bc, in0=hcol, scalar1=-0.5, scalar2=None,
                                    op0=OP.mult)
            nc.vector.tensor_scalar(out=b1, in0=Bc, scalar1=MAGIC, scalar2=MAGIC,
                                    op0=OP.add, op1=OP.subtract)
            nc.vector.tensor_tensor(out=rr, in0=Bc, in1=b1, op=OP.subtract)
            nc.vector.tensor_scalar(out=b2, in0=rr, scalar1=256.0, scalar2=MAGIC,
                                    op0=OP.mult, op1=OP.add)
            nc.vector.tensor_scalar(out=b2, in0=b2, scalar1=MAGIC, scalar2=1.0 / 256.0,
                                    op0=OP.subtract, op1=OP.mult)
            nc.vector.tensor_tensor(out=b3, in0=rr, in1=b2, op=OP.subtract)
            for i, bb in enumerate((b1, b2, b3)):
                nc.sync.dma_start(bscr[i].rearrange("(t p) -> p t", p=128), bb)
                nc.gpsimd.dma_start(EAUG[64 + i:65 + i, :],
                                    bscr[i].rearrange("(o k) -> o k", o=1))  # cast to bf16

            for b in range(n_tiles // 8):
                ps = pt.tile([128, 1024], F32, tag="ps")
                for i in range(8):
                    j = b * 8 + i
                    xs = XF[:, j * D:(j + 1) * D]
                    nc.sync.dma_start(xs, x[j * 128:(j + 1) * 128, :])
                    nc.tensor.transpose(ps[0:64, i * 128:(i + 1) * 128], xs, IDENT)
                sl = slice(b * 1024, (b + 1) * 1024)
                psx = ps[0:64, :]
                nc.vector.tensor_copy(T1[0:64, sl], psx)            # xh.T bf16
                nc.vector.tensor_copy(T2[0:64, sl], psx)
                xl16 = ep.tile([64, 1024], BF16, tag="xl16")
                nc.vector.tensor_tensor(out=xl16, in0=psx, in1=T1[0:64, sl],
                                        op=OP.subtract)
                nc.sync.dma_start(T2[64:128, sl], xl16)             # partition move

        # ---------------- main loop ----------------
        with ExitStack() as mctx:
            pm = mctx.enter_context(tc.tile_pool(name="pm", bufs=4, space="PSUM"))
            cp = mctx.enter_context(tc.tile_pool(name="cp", bufs=3))
            mp = mctx.enter_context(tc.tile_pool(name="mp", bufs=3))
            sp = mctx.enter_context(tc.tile_pool(name="sp", bufs=3))
            bp = mctx.enter_context(tc.tile_pool(name="bp", bufs=3))

            for b in range(n_tiles // 8):
                nh8 = bp.tile([128, 8], F32, tag="nh8")
                n32 = bp.tile([128, 8], U32, tag="n32")
                EP = bp.tile([128, 1024], F32, tag="EP")
                for i in range(8):
                    j = b * 8 + i
                    l1 = T1[:, j * 128:(j + 1) * 128]
                    l2 = T2[:, j * 128:(j + 1) * 128]
                    C = cp.tile([128, 2048], F32, tag="C")
                    max1 = mp.tile([128, 2048], F32, tag="max1")
                    scrap = sp.tile([128, 2048], F32, tag="scrap")
                    mst = sp.tile([128, 1], F32, tag="mst")
                    for t in range(4):
                        pst = pm.tile([128, 1024], F32, tag="pst")
                        for h in range(2):
                            c = 2 * t + h
                            psc = pst[:, h * 512:(h + 1) * 512]
                            nc.tensor.matmul(psc, l1, EAUG[:, c * 512:(c + 1) * 512],
                                             start=True, stop=False)
                            nc.tensor.matmul(psc, l2, E2[:, c * 512:(c + 1) * 512],
                                             start=False, stop=True)
                        sl = slice(t * 512, (t + 1) * 512)
                        evens = pst.rearrange("p (w two) -> p w two", two=2)[:, :, 0]
                        odds = pst.rearrange("p (w two) -> p w two", two=2)[:, :, 1]
                        nc.scalar.copy(C[:, sl], odds)
                        nc.vector.tensor_tensor(out=max1[:, sl], in0=evens,
                                                in1=C[:, sl], op=OP.max)
                    nc.vector.tensor_reduce(out=mst, in_=max1, axis=AX.X, op=OP.max)
                    nc.vector.scalar_tensor_tensor(
                        out=scrap, in0=max1, scalar=mst, in1=IOTA,
                        op0=OP.is_ge, op1=OP.mult, accum_out=nh8[:, i:i + 1])
                    nc.vector.tensor_copy(n32[:, i:i + 1], nh8[:, i:i + 1])
                    nc.gpsimd.indirect_dma_start(
                        out=EP[:, i * 128:(i + 1) * 128], out_offset=None, in_=etab,
                        in_offset=IndirectOffsetOnAxis(ap=n32[:, i:i + 1], axis=0),
                        bounds_check=K - 1, oob_is_err=False)

                # ---- batched postlude: pair resolve + outputs ----
                xsl = XF[:, b * 512:(b + 1) * 512]
                EPv = EP.rearrange("p (j two d) -> p j two d", two=2, d=D)
                D0 = bp.tile([128, 8], F32, tag="D0")
                D1 = bp.tile([128, 8], F32, tag="D1")
                for half, Dc in ((0, D0), (1, D1)):
                    Ec = EPv[:, :, half, :]
                    DF = bp.tile([128, 512], F32, tag="DF")
                    SQ = bp.tile([128, 512], F32, tag="SQ")
                    nc.vector.tensor_tensor(out=DF, in0=xsl, in1=Ec, op=OP.subtract)
                    nc.vector.tensor_tensor(out=SQ, in0=DF, in1=DF, op=OP.mult)
                    nc.vector.tensor_reduce(
                        out=Dc, in_=SQ.rearrange("p (j d) -> p j d", d=D),
                        axis=AX.X, op=OP.add)
                BIT = bp.tile([128, 8], F32, tag="BIT")
                KSF = bp.tile([128, 8], F32, tag="KSF")
                KS32 = bp.tile([128, 8], I32, tag="KS32")
                KSU = bp.tile([128, 8], U32, tag="KSU")
                nc.vector.tensor_tensor(out=BIT, in0=D0, in1=D1, op=OP.is_gt)
                nc.vector.tensor_tensor(out=KSF, in0=BIT, in1=nh8, op=OP.add)
                nc.vector.tensor_copy(KS32, KSF)
                nc.vector.tensor_copy(KSU, KSF)
                QT = bp.tile([128, 512], F32, tag="QT")
                for i in range(8):
                    nc.gpsimd.indirect_dma_start(
                        out=QT[:, i * D:(i + 1) * D], out_offset=None, in_=etab,
                        in_offset=IndirectOffsetOnAxis(ap=KSU[:, i:i + 1], axis=0),
                        bounds_check=K - 1, oob_is_err=False)
                qsl = quant[b * 1024:(b + 1) * 1024, :].rearrange(
                    "(j p) d -> p j d", p=128)
                nc.sync.dma_start(qsl, QT)
                isl = eidx[b * 1024:(b + 1) * 1024, :].rearrange(
                    "(j p) o -> p j o", p=128)
                nc.sync.dma_start(isl, KS32)

    nc.compile()
    return nc


_CACHE = {}


def _get_nc():
    if "nc" not in _CACHE:
        nc = bacc.Bacc("TRN2", target_bir_lowering=False, debug=False,
                       num_devices=NCORES)
        _CACHE["nc"] = build(nc)
    return _CACHE["nc"]


def kernel(x: np.ndarray, embed: np.ndarray):
    nc = _get_nc()
    x = np.ascontiguousarray(np.asarray(x), dtype=np.float32)
    embed = np.ascontiguousarray(np.asarray(embed), dtype=np.float32)
    in_maps = [
        {"x": x[c * NS:(c + 1) * NS], "embed": embed}
        for c in range(NCORES)
    ]
    res = run_bass_kernel_spmd(nc, in_maps, list(range(NCORES)))
    quant = np.concatenate([res.results[c]["quant"] for c in range(NCORES)], axis=0)
    idx = np.concatenate([res.results[c]["eidx"][:, 0] for c in range(NCORES)], axis=0)
    return quant, idx[None, :].astype(np.int32)


# revision 13
# speedup vs baseline: 1.0574x; 1.0574x over previous
"""EuclideanCodebook (VQ) kernel for 8 Trainium2 NeuronCores.

Math: argmin_k ||x - e_k||^2 == argmax_k s_k,  s_k = x.e_k - ||e_k||^2/2.
Scores are computed at ~fp32 precision on the PE via a hi/lo bf16 split:
  s = xh.eh + (b1+b2+b3) [MM1, K=67]  +  xh.el + xl.eh [MM2, K=128]
where b1,b2,b3 are bf16-exact digits of -||e||^2/2.

Argmax per 128-token tile over 4096 codes:
  - 8 matmul chunks land in PSUM [128, 512] each.
  - ACT copies chunks 4..7 to SBUF (C), DVE tensor_tensor_reduce chain
    computes max1 = max(psum_c, C_c) elementwise (pairs k, k+2048) plus
    the row max m* via chained accumulators.
  - One scalar_tensor_tensor pass: sum((max1 >= m*) * iota) = pair slot n.
  - Pair resolved by gathering e[n], e[n+2048] via indirect DMA and
    rescoring exact squared distances on DVE (also the tie-breaker).
quantize rows are gathered from the embed table by the final index.

Sharding: x split into 8 shards of 8192 tokens (one per core); embed
replicated. SPMD program via run_bass_kernel_spmd; host only
concatenates shard outputs.
"""

from contextlib import ExitStack

import numpy as np

import concourse.bass as bass
import concourse.bacc as bacc
import concourse.tile as tile
from concourse import masks, mybir
from concourse.bass import IndirectOffsetOnAxis
from concourse.bass_utils import run_bass_kernel_spmd

F32 = mybir.dt.float32
BF16 = mybir.dt.bfloat16
I32 = mybir.dt.int32
U32 = mybir.dt.uint32
OP = mybir.AluOpType
AX = mybir.AxisListType

N, K, D = 65536, 4096, 64
NCORES = 8
NS = N // NCORES          # 8192 tokens per core
NT = NS // 128            # 64 token tiles
ET_TILES = K // 128       # 32 embed tiles
NEG = -3.0e38


def build(nc: bacc.Bacc, n_tiles: int = NT):
    x = nc.dram_tensor("x", [NS, D], F32, kind="ExternalInput").ap()
    embed = nc.dram_tensor("embed", [1, K, D], F32, kind="ExternalInput").ap()
    quant = nc.dram_tensor("quant", [NS, D], F32, kind="ExternalOutput").ap()
    eidx = nc.dram_tensor("eidx", [NS, 1], I32, kind="ExternalOutput").ap()
    bscr = nc.dram_tensor("bscr", [3, K], F32).ap()
    etab = embed.rearrange("o k d -> (o k) d")  # [K, D], offset 0

    # persistent sbuf
    XF = nc.alloc_sbuf_tensor("XF", [128, NT * D], F32).ap()
    T1 = nc.alloc_sbuf_tensor("T1", [67, NS], BF16).ap()
    T2 = nc.alloc_sbuf_tensor("T2", [128, NS], BF16).ap()
    EAUG = nc.alloc_sbuf_tensor("EAUG", [67, K], BF16).ap()
    E2 = nc.alloc_sbuf_tensor("E2", [128, K], BF16).ap()
    IOTA = nc.alloc_sbuf_tensor("IOTA", [128, K // 4], F32).ap()
    IOTI = nc.alloc_sbuf_tensor("IOTI", [128, K // 4], I32).ap()
    IDENT = nc.alloc_sbuf_tensor("IDENT", [128, 128], F32).ap()

    with tile.TileContext(nc) as tc:
        masks.make_identity(nc, IDENT)
        nc.gpsimd.memset(T1[64:67, :], 1.0)
        nc.gpsimd.iota(IOTI, pattern=[[1024, 2], [2, 512]], base=0, channel_multiplier=0)
        nc.vector.tensor_copy(IOTA, IOTI)

        # ---------------- prep: embed + x (own pool scope) ----------------
        with ExitStack() as pctx:
            ep = pctx.enter_context(tc.tile_pool(name="ep", bufs=3))
            pt = pctx.enter_context(tc.tile_pool(name="pt", bufs=2, space="PSUM"))

            hcol = nc.alloc_sbuf_tensor("hcol", [128, ET_TILES], F32).ap()
            for bt in range(ET_TILES // 8):
                ps = pt.tile([128, 1024], F32, tag="ps")
                for i in range(8):
                    t = bt * 8 + i
                    etile = ep.tile([128, D], F32, tag="etile")
                    scr64 = ep.tile([128, D], F32, tag="scr64")
                    nc.sync.dma_start(etile, etab[t * 128:(t + 1) * 128, :])
                    nc.vector.tensor_tensor(out=scr64, in0=etile, in1=etile,
                                            op=OP.mult)
                    nc.vector.tensor_reduce(out=hcol[:, t:t + 1], in_=scr64,
                                            axis=AX.X, op=OP.add)
                    nc.tensor.transpose(ps[0:64, i * 128:(i + 1) * 128], etile, IDENT)
                sl = slice(bt * 1024, (bt + 1) * 1024)
                pse = ps[0:64, :]
                nc.vector.tensor_copy(EAUG[0:64, sl], pse)          # eh.T bf16
                nc.vector.tensor_tensor(out=E2[0:64, sl], in0=pse,
                                        in1=EAUG[0:64, sl], op=OP.subtract)  # el.T
                eh16 = ep.tile([64, 1024], BF16, tag="eh16")
                nc.vector.tensor_copy(eh16, pse)
                nc.sync.dma_start(E2[64:128, sl], eh16)             # partition move

            # bias digits: B = -h/2; b1 = round(B); r = B-b1;
            # b2 = round(r*256)/256; b3 = r-b2
            MAGIC = float(2 ** 23)
            Bc = ep.tile([128, ET_TILES], F32, tag="Bc")
            b1 = ep.tile([128, ET_TILES], F32, tag="b1")
            b2 = ep.tile([128, ET_TILES], F32, tag="b2")
            b3 = ep.tile([128, ET_TILES], F32, tag="b3")
            rr = ep.tile([128, ET_TILES], F32, tag="rr")
            nc.vector.tensor_scalar(out=Bc, in0=hcol, scalar1=-0.5, scalar2=None,
                                    op0=OP.mult)
            nc.vector.tensor_scalar(out=b1, in0=Bc, scalar1=MAGIC, scalar2=MAGIC,
                                    op0=OP.add, op1=OP.subtract)
            nc.vector.tensor_tensor(out=rr, in0=Bc, in1=b1, op=OP.subtract)
            nc.vector.tensor_scalar(out=b2, in0=rr, scalar1=256.0, scalar2=MAGIC,
                                    op0=OP.mult, op1=OP.add)
            nc.vector.tensor_scalar(out=b2, in0=b2, scalar1=MAGIC, scalar2=1.0 / 256.0,
                                    op0=OP.subtract, op1=OP.mult)
            nc.vector.tensor_tensor(out=b3, in0=rr, in1=b2, op=OP.subtract)
            for i, bb in enumerate((b1, b2, b3)):
                nc.sync.dma_start(bscr[i].rearrange("(t p) -> p t", p=128), bb)
                nc.gpsimd.dma_start(EAUG[64 + i:65 + i, :],
                                    bscr[i].rearrange("(o k) -> o k", o=1))  # cast to bf16

            for b in range(n_tiles // 8):
                ps = pt.tile([128, 1024], F32, tag="ps")
                for i in range(8):
                    j = b * 8 + i
                    xs = XF[:, j * D:(j + 1) * D]
                    nc.sync.dma_start(xs, x[j * 128:(j + 1) * 128, :])
                    nc.tensor.transpose(ps[0:64, i * 128:(i + 1) * 128], xs, IDENT)
                sl = slice(b * 1024, (b + 1) * 1024)
                psx = ps[0:64, :]
                nc.vector.tensor_copy(T1[0:64, sl], psx)            # xh.T bf16
                nc.vector.tensor_copy(T2[0:64, sl], psx)
                xl16 = ep.tile([64, 1024], BF16, tag="xl16")
                nc.vector.tensor_tensor(out=xl16, in0=psx, in1=T1[0:64, sl],
                                        op=OP.subtract)
                nc.sync.dma_start(T2[64:128, sl], xl16)             # partition move

        # ---------------- main loop ----------------
        with ExitStack() as mctx:
            pm = mctx.enter_context(tc.tile_pool(name="pm", bufs=4, space="PSUM"))
            cp = mctx.enter_context(tc.tile_pool(name="cp", bufs=3))
            mp = mctx.enter_context(tc.tile_pool(name="mp", bufs=3))
            sp = mctx.enter_context(tc.tile_pool(name="sp", bufs=3))
            bp = mctx.enter_context(tc.tile_pool(name="bp", bufs=3))

            for b in range(n_tiles // 8):
                nh8 = bp.tile([128, 8], F32, tag="nh8")
                n32 = bp.tile([128, 8], U32, tag="n32")
                EP = bp.tile([128, 2048], F32, tag="EP")
                for i in range(8):
                    j = b * 8 + i
                    l1 = T1[:, j * 128:(j + 1) * 128]
                    l2 = T2[:, j * 128:(j + 1) * 128]
                    C = cp.tile([128, 2048], F32, tag="C")
                    max1 = mp.tile([128, 2048], F32, tag="max1")
                    scrap = sp.tile([128, 2048], F32, tag="scrap")
                    mst = sp.tile([128, 1], F32, tag="mst")
                    for t in range(4):
                        pst = pm.tile([128, 1024], F32, tag="pst")
                        for h in range(2):
                            c = 2 * t + h
                            psc = pst[:, h * 512:(h + 1) * 512]
                            nc.tensor.matmul(psc, l1, EAUG[:, c * 512:(c + 1) * 512],
                                             start=True, stop=False)
                            nc.tensor.matmul(psc, l2, E2[:, c * 512:(c + 1) * 512],
                                             start=False, stop=True)
                        sl = slice(t * 512, (t + 1) * 512)
                        evens = pst.rearrange("p (w two) -> p w two", two=2)[:, :, 0]
                        odds = pst.rearrange("p (w two) -> p w two", two=2)[:, :, 1]
                        nc.scalar.copy(C[:, sl], odds)
                        nc.vector.tensor_tensor(out=max1[:, sl], in0=evens,
                                                in1=C[:, sl], op=OP.max)
                    max2 = mp.tile([128, 1024], F32, tag="max2")
                    nc.vector.tensor_tensor(out=max2, in0=max1[:, 0:1024],
                                            in1=max1[:, 1024:2048], op=OP.max)
                    nc.vector.tensor_reduce(out=mst, in_=max2, axis=AX.X, op=OP.max)
                    nc.vector.scalar_tensor_tensor(
                        out=scrap[:, 0:1024], in0=max2, scalar=mst, in1=IOTA,
                        op0=OP.is_ge, op1=OP.mult, accum_out=nh8[:, i:i + 1])
                    nc.vector.tensor_copy(n32[:, i:i + 1], nh8[:, i:i + 1])
                    nc.gpsimd.indirect_dma_start(
                        out=EP[:, i * 256:i * 256 + 128], out_offset=None, in_=etab,
                        in_offset=IndirectOffsetOnAxis(ap=n32[:, i:i + 1], axis=0),
                        bounds_check=K - 1, oob_is_err=False)
                    nc.gpsimd.indirect_dma_start(
                        out=EP[:, i * 256 + 128:i * 256 + 256], out_offset=None,
                        in_=etab,
                        in_offset=IndirectOffsetOnAxis(ap=n32[:, i:i + 1], axis=0),
                        element_offset=2048 * D,
                        bounds_check=K - 1, oob_is_err=False)

                # ---- batched postlude: pair resolve + outputs ----
                xsl = XF[:, b * 512:(b + 1) * 512]
                EPv = EP.rearrange("p (j c d) -> p j c d", c=4, d=D)
                DC = [bp.tile([128, 8], F32, name=f"Dc{c}", tag=f"D{c}") for c in range(4)]
                for c in range(4):
                    Ec = EPv[:, :, c, :]
                    DF = bp.tile([128, 512], F32, tag="DF")
                    SQ = bp.tile([128, 512], F32, tag="SQ")
                    nc.gpsimd.tensor_tensor(out=DF, in0=xsl, in1=Ec, op=OP.subtract)
                    nc.gpsimd.tensor_tensor(out=SQ, in0=DF, in1=DF, op=OP.mult)
                    nc.vector.tensor_reduce(
                        out=DC[c], in_=SQ.rearrange("p (j d) -> p j d", d=D),
                        axis=AX.X, op=OP.add)
                # candidates: kA0=nh, kA1=nh+1, kB0=nh+2048, kB1=nh+2049
                BA = bp.tile([128, 8], F32, tag="BA")
                BB = bp.tile([128, 8], F32, tag="BB")
                MA = bp.tile([128, 8], F32, tag="MA")
                MB = bp.tile([128, 8], F32, tag="MB")
                BF = bp.tile([128, 8], F32, tag="BF")
                KA = bp.tile([128, 8], F32, tag="KA")
                KB = bp.tile([128, 8], F32, tag="KB")
                DK = bp.tile([128, 8], F32, tag="DK")
                KSF = bp.tile([128, 8], F32, tag="KSF")
                KS32 = bp.tile([128, 8], I32, tag="KS32")
                KSU = bp.tile([128, 8], U32, tag="KSU")
                nc.vector.tensor_tensor(out=BA, in0=DC[0], in1=DC[1], op=OP.is_gt)
                nc.vector.tensor_tensor(out=BB, in0=DC[2], in1=DC[3], op=OP.is_gt)
                nc.vector.tensor_tensor(out=MA, in0=DC[0], in1=DC[1], op=OP.min)
                nc.vector.tensor_tensor(out=MB, in0=DC[2], in1=DC[3], op=OP.min)
                nc.vector.tensor_tensor(out=BF, in0=MA, in1=MB, op=OP.is_gt)
                nc.vector.tensor_tensor(out=KA, in0=nh8, in1=BA, op=OP.add)
                nc.vector.tensor_scalar(out=KB, in0=nh8, scalar1=2048.0,
                                        scalar2=None, op0=OP.add)
                nc.vector.tensor_tensor(out=KB, in0=KB, in1=BB, op=OP.add)
                nc.vector.tensor_tensor(out=DK, in0=KB, in1=KA, op=OP.subtract)
                nc.vector.tensor_tensor(out=DK, in0=DK, in1=BF, op=OP.mult)
                nc.vector.tensor_tensor(out=KSF, in0=KA, in1=DK, op=OP.add)
                nc.vector.tensor_copy(KS32, KSF)
                nc.vector.tensor_copy(KSU, KSF)
                QT = bp.tile([128, 512], F32, tag="QT")
                for i in range(8):
                    nc.gpsimd.indirect_dma_start(
                        out=QT[:, i * D:(i + 1) * D], out_offset=None, in_=etab,
                        in_offset=IndirectOffsetOnAxis(ap=KSU[:, i:i + 1], axis=0),
                        bounds_check=K - 1, oob_is_err=False)
                qsl = quant[b * 1024:(b + 1) * 1024, :].rearrange(
                    "(j p) d -> p j d", p=128)
                nc.sync.dma_start(qsl, QT)
                isl = eidx[b * 1024:(b + 1) * 1024, :].rearrange(
                    "(j p) o -> p j o", p=128)
                nc.sync.dma_start(isl, KS32)

    nc.compile()
    return nc


_CACHE = {}


def _get_nc():
    if "nc" not in _CACHE:
        nc = bacc.Bacc("TRN2", target_bir_lowering=False, debug=False,
                       num_devices=NCORES)
        _CACHE["nc"] = build(nc)
    return _CACHE["nc"]


def kernel(x: np.ndarray, embed: np.ndarray):
    nc = _get_nc()
    x = np.ascontiguousarray(np.asarray(x), dtype=np.float32)
    embed = np.ascontiguousarray(np.asarray(embed), dtype=np.float32)
    in_maps = [
        {"x": x[c * NS:(c + 1) * NS], "embed": embed}
        for c in range(NCORES)
    ]
    res = run_bass_kernel_spmd(nc, in_maps, list(range(NCORES)))
    quant = np.concatenate([res.results[c]["quant"] for c in range(NCORES)], axis=0)
    idx = np.concatenate([res.results[c]["eidx"][:, 0] for c in range(NCORES)], axis=0)
    return quant, idx[None, :].astype(np.int32)
